# revision 30
# baseline (speedup 1.0000x reference)
"""PointNet (2x PointNetConv + global max pool + linear head) on 8 TRN2 cores.

Data-parallel over the 8 graphs: core c owns graph c (1024 nodes, 16-NN).
Per-core pipeline entirely on-chip:
  kNN via matmul + vector.max/max_index/match_replace (top-16)
  conv1/conv2 via linearity: h[e] = T[j_e] - C[i_e], T gathered with
  dma_gather (hi/lo fp16 split for near-fp32 precision), BN stats
  AllReduduced across cores, edge-max == node-max o k-max for pooling.
All pre-BN biases cancel (BN absorbs constant shifts) and are dropped.
"""

import os

import numpy as np

import concourse.bacc as bacc
import concourse.mybir as mybir
import concourse.tile as tile
from concourse import bass_utils
from concourse.masks import make_identity

dt = mybir.dt
F32, F16, I16, U16 = dt.float32, dt.float16, dt.int16, dt.uint16
AF = mybir.ActivationFunctionType
ALU = mybir.AluOpType
AX = mybir.AxisListType

B = 8
N = 1024
K = 16
E = N * K  # 16384 edges per core
NCORES = 8
EPS = 1e-5
RG = [list(range(NCORES))]
_STAGE = int(os.environ.get("KSTAGE", "99"))


class _Stop(Exception):
    pass


def _emit(nc, tc, d, out_d):
    # ---------------- persistent tiles ----------------
    # Keep free fns alive: dropping them GC-releases the pool mid-trace,
    # letting later pools reuse the same SBUF zone (AddressConflictError).
    frees = []

    def T(shape, dtype, name):
        ap, f = tc.tile(shape, dtype, name=name)
        frees.append(f)
        return ap

    ident = T([128, 128], F32, "ident")
    fold = T([128, 64], F16, "fold")
    posT_sb = T([3, N], F32, "posT_sb")
    pos16 = T([3, N], F16, "pos16")
    poslo = T([3, N], F16, "poslo")
    rhs4 = T([4, N], F32, "rhs4")
    lhsT4 = T([4, N], F32, "lhsT4")
    sqpos = T([1, N], F32, "sqpos")
    possq = T([3, N], F32, "possq")
    ones3 = T([3, 1], F32, "ones3")
    w1sum_sb = T([3, 64], F32, "w1sum_sb")
    w1b_sb = T([3, 64], F32, "w1b_sb")
    w21p_sb = T([3, 64], F32, "w21p_sb")
    w2_sb = T([64, 64], F16, "w2_sb")
    w21x_sb = T([64, 64], F16, "w21x_sb")
    w21p16_sb = T([3, 64], F16, "w21p16_sb")
    w22_sb = T([64, 128], F16, "w22_sb")
    w3_sb = T([128, 1024], F16, "w3_sb")
    linw_sb = T([128, 2048], F32, "linw_sb")  # [p, kc*256 + m]
    gb1_sb = T([64, 2], F32, "gb1_sb")
    gb21_sb = T([64, 2], F32, "gb21_sb")
    gb22_sb = T([128, 2], F32, "gb22_sb")
    gbf_sb = T([128, 4], F32, "gbf_sb")
    packed32 = T([128, 128], dt.uint32, "packed32")
    pf32 = T([128, 64], F32, "pf32")
    TtI16 = T([64, 128], I16, "TtI16")
    idxw = T([128, 1024], I16, "idxw")
    A_sb = T([64, N], F32, "A_sb")
    C1_sb = T([64, N], F32, "C1_sb")
    C2_sb = T([64, N], F32, "C2_sb")
    G2_sb = T([64, N], F32, "G2_sb")
    NT_hi = T([128, 512], F16, "NT_hi")  # [p, c0*64 + ch]
    NT_lo = T([128, 512], F16, "NT_lo")
    x1m = T([64, N], F32, "x1m")
    x1hi = T([64, N], F16, "x1hi")
    x1lo = T([64, N], F16, "x1lo")
    gats = [T([128, 512], F16, f"gat{c}") for c in range(32)]
    gc8 = T([128, 1024], F16, "gc8")  # [p, (oc*16+eb)*8 + j]
    g_sb = T([128, 8], F32, "g_sb")
    go_sb = T([8, 1024], F32, "go_sb")
    gall_sb = T([128, 64], F32, "gall_sb")  # [p, kc*8 + graph]
    bnst = T([128, 192], F32, "bnst")  # 32 blocks x 6
    st2 = T([128, 2], F32, "st2")
    st_in = T([128, 2], F32, "st_in")
    st_g = T([128, 2], F32, "st_g")
    bnw = T([128, 10], F32, "bnw")
    qdump = T([128, 8], F32, "qdump")
    outf_sb = T([128, 8], F32, "outf_sb")
    osb = T([8, 128], F32, "osb")
    v8a = T([128, 8], F32, "v8a")
    v8b = T([128, 8], F32, "v8b")
    epsc = T([128, 1], F32, "epsc")
    hA = T([128, E], F16, "hA")
    hB = T([128, E], F16, "hB")

    def ck(stage):
        if _STAGE <= stage:
            raise _Stop

    try:
        with tc.tile_pool(name="scr", bufs=3) as scr, \
             tc.tile_pool(name="psb", bufs=3, space="PSUM") as ps_big, \
             tc.tile_pool(name="pss", bufs=2, space="PSUM") as ps_small, \
             tc.tile_pool(name="dtab", bufs=1, space="DRAM") as dtab, \
             tc.tile_pool(name="dcc", bufs=1, space="DRAM") as dcc:

            a_tab = dtab.tile([N, 128], F16, name="a_tab")
            g2_tab = dtab.tile([N, 128], F16, name="g2_tab")
            pk_d = dtab.tile([128, 128], I16, name="pk_d")
            cc1i = dcc.tile([64, 2], F32, name="cc1i")
            cc1o = dcc.tile([64, 2], F32, name="cc1o")
            cc2i = dcc.tile([64, 2], F32, name="cc2i")
            cc2o = dcc.tile([64, 2], F32, name="cc2o")
            cc3i = dcc.tile([128, 2], F32, name="cc3i")
            cc3o = dcc.tile([128, 2], F32, name="cc3o")
            g_inh = [dcc.tile([512], F32, name=f"g_in{h}") for h in range(2)]
            g_outh = [dcc.tile([B, 512], F32, name=f"g_out{h}")
                      for h in range(2)]

            # ---------------- load weights ----------------
            nc.sync.dma_start(out=posT_sb[:], in_=d["posT"][:])
            nc.sync.dma_start(out=w1sum_sb[:], in_=d["w1sum"][:])
            nc.sync.dma_start(out=w1b_sb[:], in_=d["w1b"][:])
            nc.sync.dma_start(out=w21p_sb[:], in_=d["w21p"][:])
            nc.sync.dma_start(out=w2_sb[:], in_=d["w2w"][:])
            nc.sync.dma_start(out=w21x_sb[:], in_=d["w21x"][:])
            nc.sync.dma_start(out=w21p16_sb[:], in_=d["w21p16"][:])
            nc.sync.dma_start(out=w22_sb[:], in_=d["w22w"][:])
            nc.sync.dma_start(out=w3_sb[:], in_=d["w3w"][:])
            nc.sync.dma_start(
                out=linw_sb[:].rearrange("p (kc m) -> p kc m", m=256),
                in_=d["linw"].rearrange("(kc p) m -> p kc m", p=128),
            )
            nc.sync.dma_start(out=gb1_sb[:], in_=d["gb1"][:])
            nc.sync.dma_start(out=gb21_sb[:], in_=d["gb21"][:])
            nc.sync.dma_start(out=gb22_sb[:], in_=d["gb22"][:])
            nc.sync.dma_start(out=gbf_sb[:], in_=d["gbf"][:])

            make_identity(nc, ident[:])
            nc.gpsimd.memset(fold[:], 0.0)
            for base in (0, -64):
                nc.gpsimd.affine_select(
                    out=fold[:], in_=fold[:], compare_op=ALU.not_equal,
                    fill=1.0, base=base, pattern=[[-1, 64]], channel_multiplier=1,
                )
            nc.vector.memset(epsc[:], EPS)

            # ---------------- pos prep ----------------
            nc.scalar.square(possq[:], posT_sb[:])
            nc.vector.memset(ones3[:], 1.0)
            for h in range(2):
                sl = slice(h * 512, (h + 1) * 512)
                sq_ps = ps_small.tile([1, 512], F32, name="small")
                nc.tensor.matmul(out=sq_ps[:], lhsT=ones3[:],
                                 rhs=possq[:, sl], start=True, stop=True)
                nc.scalar.copy(sqpos[:, sl], sq_ps[:])
            nc.scalar.copy(rhs4[0:3, :], posT_sb[:])
            nc.sync.dma_start(out=rhs4[3:4, :], in_=sqpos[:])
            nc.vector.memset(lhsT4[:], -1.0)
            nc.scalar.mul(lhsT4[0:3, :], posT_sb[:], 2.0)
            nc.scalar.copy(pos16[:], posT_sb[:])
            nc.vector.tensor_tensor(out=poslo[:], in0=posT_sb[:], in1=pos16[:],
                                    op=ALU.subtract)
            ck(1)

            # ---------------- kNN top-16 ----------------
            # score[i,j] = 2 p_i.p_j - |p_j|^2  (row-constant -|p_i|^2 dropped)
            # idxw is built per-HALF so conv1 gathers for the first 512 nodes
            # can start while the second half of the top-k still runs:
            # u32 idx -> f32 (exact) -> PE transpose -> i16 -> DRAM bounce ->
            # one strided read per gpsimd core group.
            for m in range(8):
                D_ps = ps_big.tile([128, N], F32, name="big")
                for h in range(2):
                    sl = slice(h * 512, (h + 1) * 512)
                    nc.tensor.matmul(out=D_ps[:, sl],
                                     lhsT=lhsT4[:, m * 128:(m + 1) * 128],
                                     rhs=rhs4[:, sl], start=True, stop=True)
                D_sb = scr.tile([128, N], F32, name="dsb")
                nc.scalar.copy(D_sb[:], D_ps[:])
                nc.vector.max(v8a[:], D_sb[:])
                nc.vector.max_index(packed32[:, m * 16:m * 16 + 8], v8a[:], D_sb[:])
                nc.vector.match_replace(D_sb[:], v8a[:], D_sb[:], -1e30)
                nc.vector.max(v8b[:], D_sb[:])
                nc.vector.max_index(packed32[:, m * 16 + 8:m * 16 + 16], v8b[:],
                                    D_sb[:])
                if m % 4 == 3:
                    hb = m // 4  # half 0: chunks 0-3, half 1: chunks 4-7
                    cs = slice(hb * 64, (hb + 1) * 64)
                    nc.scalar.copy(pf32[:], packed32[:, cs].bitcast(mybir.dt.int32))
                    pT2 = ps_small.tile([64, 128], F32, name="small")
                    nc.tensor.transpose(pT2[:], pf32[:], ident[:])
                    nc.scalar.copy(TtI16[:], pT2[:])
                    nc.sync.dma_start(out=pk_d[cs, :], in_=TtI16[:])
                    for g in range(8):
                        nc.sync.dma_start(
                            out=idxw[g * 16:(g + 1) * 16,
                                     hb * 512:(hb + 1) * 512].rearrange(
                                         "k (m p) -> k m p", m=4),
                            in_=pk_d[cs, :].rearrange("(m k) p -> k m p", k=16))
            ck(2)

            # ---------------- A / C1 / C2 node features ----------------
            def build_ac(dst, lhsT_w, nm):
                ps = ps_big.tile([64, N], F32, name="big")
                for h in range(2):
                    sl = slice(h * 512, (h + 1) * 512)
                    nc.tensor.matmul(out=ps[:, sl], lhsT=lhsT_w[:],
                                     rhs=posT_sb[:, sl], start=True, stop=True)
                nc.scalar.copy(dst[:], ps[:])

            build_ac(A_sb, w1sum_sb, "A_ps")
            build_ac(C1_sb, w1b_sb, "C1_ps")
            build_ac(C2_sb, w21p_sb, "C2_ps")

            def build_table(src_sb, tab, nm):
                # tab[j, 0:64] = f16 hi of src[:, j]; tab[j, 64:128] = f16 lo
                for c0 in range(8):
                    pT = ps_small.tile([128, 64], F32, name="small")
                    nc.tensor.transpose(pT[:], src_sb[:, c0 * 128:(c0 + 1) * 128],
                                        ident[0:64, 0:64])
                    hi = NT_hi[:, c0 * 64:(c0 + 1) * 64]
                    nc.scalar.copy(hi, pT[:])
                    nc.vector.tensor_tensor(out=NT_lo[:, c0 * 64:(c0 + 1) * 64],
                                            in0=pT[:], in1=hi, op=ALU.subtract)
                tabv = tab.rearrange("(c p) ch -> p c ch", p=128)
                hiv = NT_hi[:].rearrange("p (c ch) -> p c ch", ch=64)
                lov = NT_lo[:].rearrange("p (c ch) -> p c ch", ch=64)
                nc.sync.dma_start(out=tabv[:, :, 0:64], in_=hiv)
                nc.sync.dma_start(out=tabv[:, :, 64:128], in_=lov)

            build_table(A_sb, a_tab, "aT")
            ck(3)

            def fold_one(et, h_t, c_sb):
                g = gats[et]
                sl = slice(et * 512, (et + 1) * 512)
                s_ps = ps_small.tile([64, 512], F32, name="small")
                nc.tensor.matmul(out=s_ps[:], lhsT=fold[:], rhs=g[:],
                                 start=True, stop=True)
                nc.vector.tensor_tensor(
                    out=h_t[:, sl].rearrange("c (i k) -> c i k", k=16),
                    in0=s_ps[:].rearrange("c (i k) -> c i k", k=16),
                    in1=c_sb[:, et * 32:(et + 1) * 32].to_broadcast([64, 32, 16]),
                    op=ALU.subtract)

            def gather_sub(tab, h_t, c_sb):
                # hw limit: <=512 idxs per dma_gather call; per-chunk tiles so
                # chunk c+1's DMA overlaps chunk c's fold matmul
                for et in range(32):
                    g = gats[et]
                    nc.gpsimd.dma_gather(
                        out_ap=g[:].rearrange("p (one e) -> p one e", one=1),
                        in_ap=tab[:],
                        idxs_ap=idxw[:, et * 32:(et + 1) * 32], num_idxs=512,
                        num_idxs_reg=512, elem_size=128, transpose=True,
                        queue_num=et % 4)
                    fold_one(et, h_t, c_sb)



            def bn_relu(h_t, out_t, P, gb_sb, cci, cco):
                # global-batch BN (AllReduce mean/E[x^2]) + relu, fused as ax+b
                for i in range(32):
                    nc.vector.bn_stats(bnst[0:P, i * 6:(i + 1) * 6],
                                       h_t[:, i * 512:(i + 1) * 512])
                nc.vector.bn_aggr(st2[0:P, :], bnst[0:P, :])
                nc.scalar.copy(st_in[0:P, 0:1], st2[0:P, 0:1])
                nc.scalar.square(st_in[0:P, 1:2], st2[0:P, 0:1])
                nc.vector.tensor_tensor(out=st_in[0:P, 1:2], in0=st_in[0:P, 1:2],
                                        in1=st2[0:P, 1:2], op=ALU.add)
                nc.sync.dma_start(out=cci[:], in_=st_in[0:P, :])
                nc.gpsimd.collective_compute("AllReduce", ALU.add,
                                             replica_groups=RG,
                                             ins=[cci.opt()], outs=[cco.opt()])
                nc.sync.dma_start(out=st_g[0:P, :], in_=cco[:])
                _bn_coeffs(P, gb_sb)
                for c in range(8):
                    sl = slice(c * 2048, (c + 1) * 2048)
                    nc.scalar.activation(out_t[:, sl], h_t[:, sl], AF.Relu,
                                         bias=bnw[0:P, 7:8], scale=bnw[0:P, 6:7])

            def _bn_coeffs(P, gb_sb):
                nc.scalar.mul(bnw[0:P, 0:1], st_g[0:P, 0:1], 1.0 / NCORES)  # m
                nc.scalar.mul(bnw[0:P, 1:2], st_g[0:P, 1:2], 1.0 / NCORES)  # q
                nc.scalar.square(bnw[0:P, 2:3], bnw[0:P, 0:1])
                nc.vector.tensor_tensor(out=bnw[0:P, 3:4], in0=bnw[0:P, 1:2],
                                        in1=bnw[0:P, 2:3], op=ALU.subtract)  # var
                nc.scalar.activation(bnw[0:P, 4:5], bnw[0:P, 3:4], AF.Sqrt,
                                     bias=epsc[0:P, 0:1], scale=1.0)
                nc.vector.reciprocal(bnw[0:P, 5:6], bnw[0:P, 4:5])
                nc.vector.tensor_tensor(out=bnw[0:P, 6:7], in0=gb_sb[0:P, 0:1],
                                        in1=bnw[0:P, 5:6], op=ALU.mult)  # scale
                nc.vector.tensor_tensor(out=bnw[0:P, 8:9], in0=bnw[0:P, 0:1],
                                        in1=bnw[0:P, 6:7], op=ALU.mult)
                nc.vector.tensor_tensor(out=bnw[0:P, 7:8], in0=gb_sb[0:P, 1:2],
                                        in1=bnw[0:P, 8:9], op=ALU.subtract)  # bias

            # ---------------- conv1 ----------------
            h1 = hA[0:64, :]
            gather_sub(a_tab, h1, C1_sb)
            ck(5)
            relu1 = hB[0:64, :]
            bn_relu(h1, relu1, 64, gb1_sb, cc1i, cc1o)
            ck(6)
            for et in range(32):
                sl = slice(et * 512, (et + 1) * 512)
                ps = ps_small.tile([64, 512], F32, name="small")
                nc.tensor.matmul(out=ps[:], lhsT=w2_sb[:], rhs=relu1[:, sl],
                                 start=True, stop=True)
                nc.vector.tensor_reduce(
                    out=x1m[:, et * 32:(et + 1) * 32],
                    in_=ps[:].rearrange("c (i k) -> c i k", k=16),
                    axis=AX.X, op=ALU.max)

            ck(7)
            # ---------------- conv2 ----------------
            nc.scalar.copy(x1hi[:], x1m[:])
            nc.vector.tensor_tensor(out=x1lo[:], in0=x1m[:], in1=x1hi[:],
                                    op=ALU.subtract)
            G2_ps = ps_big.tile([64, N], F32, name="big")
            for h in range(2):
                sl = slice(h * 512, (h + 1) * 512)
                nc.tensor.matmul(out=G2_ps[:, sl], lhsT=w21x_sb[:],
                                 rhs=x1hi[:, sl], start=True, stop=False)
                nc.tensor.matmul(out=G2_ps[:, sl], lhsT=w21x_sb[:],
                                 rhs=x1lo[:, sl], start=False, stop=False)
                nc.tensor.matmul(out=G2_ps[:, sl], lhsT=w21p16_sb[:],
                                 rhs=pos16[:, sl], start=False, stop=False)
                nc.tensor.matmul(out=G2_ps[:, sl], lhsT=w21p16_sb[:],
                                 rhs=poslo[:, sl], start=False, stop=True)
            nc.scalar.copy(G2_sb[:], G2_ps[:])
            build_table(G2_sb, g2_tab, "gT")
            ck(8)

            h21 = hA[0:64, :]
            gather_sub(g2_tab, h21, C2_sb)
            relu21 = hB[0:64, :]
            bn_relu(h21, relu21, 64, gb21_sb, cc2i, cc2o)

            h22 = hA[:]
            for et in range(32):
                sl = slice(et * 512, (et + 1) * 512)
                ps = ps_small.tile([128, 512], F32, name="small")
                nc.tensor.matmul(out=ps[:], lhsT=w22_sb[:], rhs=relu21[:, sl],
                                 start=True, stop=True)
                nc.scalar.copy(h22[:, sl], ps[:])
            relu22 = hB[:]
            bn_relu(h22, relu22, 128, gb22_sb, cc3i, cc3o)
            ck(9)

            # ---------------- conv2 L3 + edge-max pool ----------------
            # [128,1024] 2-bank PSUM chunks, max-reduced straight from PSUM.
            # The g AllGather + final-linear accumulation are split in two so
            # the first half overlaps the second half of the L3 matmuls.
            def ag_half(h2):
                for oc in range(h2 * 4, h2 * 4 + 4):
                    nc.vector.tensor_reduce(
                        out=g_sb[:, oc:oc + 1],
                        in_=gc8[:, oc * 128:(oc + 1) * 128],
                        axis=AX.X, op=ALU.max)
                nc.sync.dma_start(
                    out=g_inh[h2].rearrange("(c p) -> p c", p=128),
                    in_=g_sb[:, h2 * 4:(h2 + 1) * 4])
                nc.gpsimd.collective_compute("AllGather", ALU.bypass,
                                             replica_groups=RG,
                                             ins=[g_inh[h2].opt()],
                                             outs=[g_outh[h2].opt()])
                nc.sync.dma_start(out=go_sb[:, h2 * 512:(h2 + 1) * 512],
                                  in_=g_outh[h2][:])
                for cl in range(4):
                    c = h2 * 4 + cl
                    pT = ps_big.tile([128, 8], F32, name="big")
                    nc.tensor.transpose(
                        pT[:], go_sb[:, c * 128:(c + 1) * 128],
                        ident[0:8, 0:8])
                    nc.scalar.copy(gall_sb[:, c * 8:(c + 1) * 8], pT[:])

            psfs = []
            for oc in range(8):
                for eb in range(16):
                    ps = ps_big.tile([128, 1024], F32, name="big")
                    for h in range(2):
                        sl = slice(eb * 1024 + h * 512,
                                   eb * 1024 + (h + 1) * 512)
                        nc.tensor.matmul(out=ps[:, h * 512:(h + 1) * 512],
                                         lhsT=w3_sb[:, oc * 128:(oc + 1) * 128],
                                         rhs=relu22[:, sl],
                                         start=True, stop=True)
                    col = oc * 16 + eb
                    nc.vector.tensor_reduce(
                        out=gc8[:, col * 8:(col + 1) * 8],
                        in_=ps[:].rearrange("p (j e) -> p j e", j=8),
                        axis=AX.X, op=ALU.max)
                if oc == 3:
                    ag_half(0)
                    for oc2 in range(2):
                        psf = ps_small.tile([128, 8], F32, name="small")
                        psfs.append(psf)
                        for kc in range(4):
                            base = kc * 256 + oc2 * 128
                            nc.tensor.matmul(
                                out=psf[:], lhsT=linw_sb[:, base:base + 128],
                                rhs=gall_sb[:, kc * 8:(kc + 1) * 8],
                                start=(kc == 0), stop=False,
                                skip_group_check=True)

            ck(10)
            ag_half(1)
            for oc2 in range(2):
                psf = psfs[oc2]
                for kc in range(4, 8):
                    base = kc * 256 + oc2 * 128
                    nc.tensor.matmul(out=psf[:],
                                     lhsT=linw_sb[:, base:base + 128],
                                     rhs=gall_sb[:, kc * 8:(kc + 1) * 8],
                                     start=False, stop=(kc == 7),
                                     skip_group_check=True)

            ck(11)
            # ---------------- final linear + local BN + relu ----------------
            for oc2 in range(2):
                psf = psfs[oc2]
                nc.vector.tensor_reduce(out=bnw[:, 9:10], in_=psf[:],
                                        axis=AX.X, op=ALU.add)
                nc.scalar.mul(bnw[:, 0:1], bnw[:, 9:10], 1.0 / B)  # m
                nc.scalar.activation(qdump[:], psf[:], AF.Square,
                                     accum_out=bnw[:, 9:10])
                nc.scalar.mul(bnw[:, 1:2], bnw[:, 9:10], 1.0 / B)  # q
                nc.scalar.square(bnw[:, 2:3], bnw[:, 0:1])
                nc.vector.tensor_tensor(out=bnw[:, 3:4], in0=bnw[:, 1:2],
                                        in1=bnw[:, 2:3], op=ALU.subtract)
                nc.scalar.activation(bnw[:, 4:5], bnw[:, 3:4], AF.Sqrt,
                                     bias=epsc[:], scale=1.0)
                nc.vector.reciprocal(bnw[:, 5:6], bnw[:, 4:5])
                nc.vector.tensor_tensor(out=bnw[:, 6:7],
                                        in0=gbf_sb[:, oc2:oc2 + 1],
                                        in1=bnw[:, 5:6], op=ALU.mult)
                nc.vector.tensor_tensor(out=bnw[:, 8:9], in0=bnw[:, 0:1],
                                        in1=bnw[:, 6:7], op=ALU.mult)
                nc.vector.tensor_tensor(out=bnw[:, 7:8],
                                        in0=gbf_sb[:, 2 + oc2:3 + oc2],
                                        in1=bnw[:, 8:9], op=ALU.subtract)
                nc.scalar.activation(outf_sb[:], psf[:], AF.Relu,
                                     bias=bnw[:, 7:8], scale=bnw[:, 6:7])
                pso = ps_big.tile([8, 128], F32, name="big")
                nc.tensor.transpose(pso[:], outf_sb[:], ident[:])
                nc.scalar.copy(osb[:], pso[:])
                nc.sync.dma_start(out=out_d[:, oc2 * 128:(oc2 + 1) * 128],
                                  in_=osb[:])

    except _Stop:
        pass
    for f in reversed(frees):
        f()


def _build():
    nc = bacc.Bacc("TRN2", target_bir_lowering=False, debug=False,
                   num_devices=NCORES, num_swdge_queues=4)
    d = {}

    def inp(name, shape, dtype):
        d[name] = nc.dram_tensor(name, shape, dtype, kind="ExternalInput").ap()

    inp("posT", [3, N], F32)
    inp("w1sum", [3, 64], F32)
    inp("w1b", [3, 64], F32)
    inp("w21p", [3, 64], F32)
    inp("w2w", [64, 64], F16)
    inp("w21x", [64, 64], F16)
    inp("w21p16", [3, 64], F16)
    inp("w22w", [64, 128], F16)
    inp("w3w", [128, 1024], F16)
    inp("linw", [1024, 256], F32)
    inp("gb1", [64, 2], F32)
    inp("gb21", [64, 2], F32)
    inp("gb22", [128, 2], F32)
    inp("gbf", [128, 4], F32)
    out_d = nc.dram_tensor("out", [B, 256], F32, kind="ExternalOutput").ap()

    with tile.TileContext(nc) as tc:
        _emit(nc, tc, d, out_d)
    nc.finalize()
    return nc


_NC = None


def _get_nc():
    global _NC
    if _NC is None:
        _NC = _build()
    return _NC


def _prepare_in_maps(inputs):
    f32 = np.float32
    f16 = np.float16
    pos = np.asarray(inputs["pos"], dtype=f32)
    c1_W1 = np.asarray(inputs["c1_W1"], dtype=f32)
    c2_W1 = np.asarray(inputs["c2_W1"], dtype=f32)
    common = {
        "w1sum": np.ascontiguousarray(c1_W1[0:3] + c1_W1[3:6]),
        "w1b": np.ascontiguousarray(c1_W1[3:6]),
        "w21p": np.ascontiguousarray(c2_W1[64:67]),
        "w2w": np.asarray(inputs["c1_W2"], dtype=f16),
        "w21x": np.ascontiguousarray(c2_W1[0:64].astype(f16)),
        "w21p16": np.ascontiguousarray(c2_W1[64:67].astype(f16)),
        "w22w": np.asarray(inputs["c2_W2"], dtype=f16),
        "w3w": np.asarray(inputs["c2_W3"], dtype=f16),
        "linw": np.asarray(inputs["lin_W"], dtype=f32),
        "gb1": np.ascontiguousarray(
            np.stack([inputs["c1_g1"], inputs["c1_be1"]], axis=1).astype(f32)),
        "gb21": np.ascontiguousarray(
            np.stack([inputs["c2_g1"], inputs["c2_be1"]], axis=1).astype(f32)),
        "gb22": np.ascontiguousarray(
            np.stack([inputs["c2_g2"], inputs["c2_be2"]], axis=1).astype(f32)),
        "gbf": np.ascontiguousarray(np.stack(
            [np.asarray(inputs["lin_g"], dtype=f32)[0:128],
             np.asarray(inputs["lin_g"], dtype=f32)[128:256],
             np.asarray(inputs["lin_be"], dtype=f32)[0:128],
             np.asarray(inputs["lin_be"], dtype=f32)[128:256]], axis=1)),
    }
    in_maps = []
    for c in range(NCORES):
        m = dict(common)
        m["posT"] = np.ascontiguousarray(pos[c * N:(c + 1) * N].T)
        in_maps.append(m)
    return in_maps


def _run(inputs, trace=False, **kw):
    return bass_utils.run_bass_kernel_spmd(
        _get_nc(), _prepare_in_maps(inputs),
        core_ids=list(range(NCORES)), trace=trace, **kw)


def kernel(**inputs):
    res = _run(inputs)
    return np.asarray(res.results[0]["out"], dtype=np.float32)



# revision 39
# speedup vs baseline: 1.1640x; 1.1640x over previous
"""PointNet (2x PointNetConv + global max pool + linear head) on 8 TRN2 cores.

Data-parallel over the 8 graphs: core c owns graph c (1024 nodes, 16-NN).
Per-core pipeline entirely on-chip:
  kNN via matmul + vector.max/max_index/match_replace (top-16)
  conv1/conv2 via linearity: h[e] = T[j_e] - C[i_e], T gathered with
  dma_gather (hi/lo fp16 split for near-fp32 precision), BN stats
  AllReduduced across cores, edge-max == node-max o k-max for pooling.
All pre-BN biases cancel (BN absorbs constant shifts) and are dropped.
"""

import os

import numpy as np

import concourse.bacc as bacc
import concourse.mybir as mybir
import concourse.tile as tile
from concourse import bass_utils
from concourse.masks import make_identity

dt = mybir.dt
F32, F16, I16, U16 = dt.float32, dt.float16, dt.int16, dt.uint16
AF = mybir.ActivationFunctionType
ALU = mybir.AluOpType
AX = mybir.AxisListType

B = 8
N = 1024
K = 16
E = N * K  # 16384 edges per core
NCORES = 8
EPS = 1e-5
RG = [list(range(NCORES))]
_STAGE = int(os.environ.get("KSTAGE", "99"))


class _Stop(Exception):
    pass


def _emit(nc, tc, d, out_d):
    # ---------------- persistent tiles ----------------
    # Keep free fns alive: dropping them GC-releases the pool mid-trace,
    # letting later pools reuse the same SBUF zone (AddressConflictError).
    frees = []

    def T(shape, dtype, name):
        ap, f = tc.tile(shape, dtype, name=name)
        frees.append(f)
        return ap

    ident = T([128, 128], F32, "ident")
    fold = T([128, 64], F16, "fold")
    posT_sb = T([3, N], F32, "posT_sb")
    pos16 = T([3, N], F16, "pos16")
    poslo = T([3, N], F16, "poslo")
    rhs4 = T([4, N], F32, "rhs4")
    lhsT4 = T([4, N], F32, "lhsT4")
    sqpos = T([1, N], F32, "sqpos")
    possq = T([3, N], F32, "possq")
    ones3 = T([3, 1], F32, "ones3")
    w1sum_sb = T([3, 64], F32, "w1sum_sb")
    w1b_sb = T([3, 64], F32, "w1b_sb")
    w21p_sb = T([3, 64], F32, "w21p_sb")
    w2_sb = T([64, 64], F16, "w2_sb")
    w2s = T([64, 64], F16, "w2s")
    w22s = T([64, 128], F16, "w22s")
    w3s = T([128, 1024], F16, "w3s")
    w21x_sb = T([64, 64], F16, "w21x_sb")
    w21p16_sb = T([3, 64], F16, "w21p16_sb")
    w22_sb = T([64, 128], F16, "w22_sb")
    w3_sb = T([128, 1024], F16, "w3_sb")
    linw_sb = T([128, 2048], F32, "linw_sb")  # [p, kc*256 + m]
    gb1_sb = T([64, 2], F32, "gb1_sb")
    gb21_sb = T([64, 2], F32, "gb21_sb")
    gb22_sb = T([128, 2], F32, "gb22_sb")
    gbf_sb = T([128, 4], F32, "gbf_sb")
    packed32 = T([128, 128], dt.uint32, "packed32")
    pf32 = T([128, 64], F32, "pf32")
    TtI16 = T([64, 128], I16, "TtI16")
    idxw = T([128, 1024], I16, "idxw")
    A_sb = T([64, N], F32, "A_sb")
    C1_sb = T([64, N], F32, "C1_sb")
    C2_sb = T([64, N], F32, "C2_sb")
    G2_sb = T([64, N], F32, "G2_sb")
    NT_hi = T([128, 512], F16, "NT_hi")  # [p, c0*64 + ch]
    NT_lo = T([128, 512], F16, "NT_lo")
    x1m = T([64, N], F32, "x1m")
    x1hi = T([64, N], F16, "x1hi")
    x1lo = T([64, N], F16, "x1lo")
    gats = [T([128, 512], F16, f"gat{c}") for c in range(32)]
    gc8 = T([128, 1024], F16, "gc8")  # [p, (oc*16+eb)*8 + j]
    g_sb = T([128, 8], F32, "g_sb")
    go_sb = T([8, 1024], F32, "go_sb")
    gall_sb = T([128, 64], F32, "gall_sb")  # [p, kc*8 + graph]
    bnst = T([128, 192], F32, "bnst")  # 32 blocks x 6
    st2 = T([128, 2], F32, "st2")
    st_in = T([128, 2], F32, "st_in")
    st_g = T([128, 2], F32, "st_g")
    bnw = T([128, 10], F32, "bnw")
    qdump = T([128, 8], F32, "qdump")
    outf_sb = T([128, 8], F32, "outf_sb")
    osb = T([8, 128], F32, "osb")
    v8a = T([128, 8], F32, "v8a")
    v8b = T([128, 8], F32, "v8b")
    epsc = T([128, 1], F32, "epsc")
    hA = T([128, E], F16, "hA")
    hB = T([128, E], F16, "hB")

    def ck(stage):
        if _STAGE <= stage:
            raise _Stop

    try:
        with tc.tile_pool(name="scr", bufs=3) as scr, \
             tc.tile_pool(name="psb", bufs=3, space="PSUM") as ps_big, \
             tc.tile_pool(name="pss", bufs=2, space="PSUM") as ps_small, \
             tc.tile_pool(name="dtab", bufs=1, space="DRAM") as dtab, \
             tc.tile_pool(name="dcc", bufs=1, space="DRAM") as dcc:

            a_tab = dtab.tile([N, 128], F16, name="a_tab")
            g2_tab = dtab.tile([N, 128], F16, name="g2_tab")
            pk_d = dtab.tile([128, 128], I16, name="pk_d")
            cc1i = dcc.tile([64, 2], F32, name="cc1i")
            cc1o = dcc.tile([64, 2], F32, name="cc1o")
            cc2i = dcc.tile([64, 2], F32, name="cc2i")
            cc2o = dcc.tile([64, 2], F32, name="cc2o")
            cc3i = dcc.tile([128, 2], F32, name="cc3i")
            cc3o = dcc.tile([128, 2], F32, name="cc3o")
            g_inh = [dcc.tile([512], F32, name=f"g_in{h}") for h in range(2)]
            g_outh = [dcc.tile([B, 512], F32, name=f"g_out{h}")
                      for h in range(2)]

            # ---------------- load weights ----------------
            nc.sync.dma_start(out=posT_sb[:], in_=d["posT"][:])
            nc.sync.dma_start(out=w1sum_sb[:], in_=d["w1sum"][:])
            nc.sync.dma_start(out=w1b_sb[:], in_=d["w1b"][:])
            nc.sync.dma_start(out=w21p_sb[:], in_=d["w21p"][:])
            nc.sync.dma_start(out=w2_sb[:], in_=d["w2w"][:])
            nc.sync.dma_start(out=w21x_sb[:], in_=d["w21x"][:])
            nc.sync.dma_start(out=w21p16_sb[:], in_=d["w21p16"][:])
            nc.sync.dma_start(out=w22_sb[:], in_=d["w22w"][:])
            nc.sync.dma_start(out=w3_sb[:], in_=d["w3w"][:])
            nc.sync.dma_start(
                out=linw_sb[:].rearrange("p (kc m) -> p kc m", m=256),
                in_=d["linw"].rearrange("(kc p) m -> p kc m", p=128),
            )
            nc.sync.dma_start(out=gb1_sb[:], in_=d["gb1"][:])
            nc.sync.dma_start(out=gb21_sb[:], in_=d["gb21"][:])
            nc.sync.dma_start(out=gb22_sb[:], in_=d["gb22"][:])
            nc.sync.dma_start(out=gbf_sb[:], in_=d["gbf"][:])

            make_identity(nc, ident[:])
            nc.gpsimd.memset(fold[:], 0.0)
            for base in (0, -64):
                nc.gpsimd.affine_select(
                    out=fold[:], in_=fold[:], compare_op=ALU.not_equal,
                    fill=1.0, base=base, pattern=[[-1, 64]], channel_multiplier=1,
                )
            nc.vector.memset(epsc[:], EPS)

            # ---------------- pos prep ----------------
            nc.scalar.square(possq[:], posT_sb[:])
            nc.vector.memset(ones3[:], 1.0)
            for h in range(2):
                sl = slice(h * 512, (h + 1) * 512)
                sq_ps = ps_small.tile([1, 512], F32, name="small")
                nc.tensor.matmul(out=sq_ps[:], lhsT=ones3[:],
                                 rhs=possq[:, sl], start=True, stop=True)
                nc.scalar.copy(sqpos[:, sl], sq_ps[:])
            nc.scalar.copy(rhs4[0:3, :], posT_sb[:])
            nc.sync.dma_start(out=rhs4[3:4, :], in_=sqpos[:])
            nc.vector.memset(lhsT4[:], -1.0)
            nc.scalar.mul(lhsT4[0:3, :], posT_sb[:], 2.0)
            nc.scalar.copy(pos16[:], posT_sb[:])
            nc.vector.tensor_tensor(out=poslo[:], in0=posT_sb[:], in1=pos16[:],
                                    op=ALU.subtract)
            ck(1)

            # ---------------- A / C1 / C2 node features + a_tab ----------
            # built BEFORE the kNN top-k so the conv1 gathers (which need
            # a_tab AND the first idxw half) can start mid-kNN.
            def build_ac(dst, lhsT_w, nm):
                ps = ps_big.tile([64, N], F32, name="big")
                for h in range(2):
                    sl = slice(h * 512, (h + 1) * 512)
                    nc.tensor.matmul(out=ps[:, sl], lhsT=lhsT_w[:],
                                     rhs=posT_sb[:, sl], start=True, stop=True)
                nc.scalar.copy(dst[:], ps[:])

            def build_table(src_sb, tab, nm):
                # tab[j, 0:64] = f16 hi of src[:, j]; tab[j, 64:128] = f16 lo
                for c0 in range(8):
                    pT = ps_small.tile([128, 64], F32, name="small")
                    nc.tensor.transpose(pT[:], src_sb[:, c0 * 128:(c0 + 1) * 128],
                                        ident[0:64, 0:64])
                    hi = NT_hi[:, c0 * 64:(c0 + 1) * 64]
                    nc.scalar.copy(hi, pT[:])
                    nc.vector.tensor_tensor(out=NT_lo[:, c0 * 64:(c0 + 1) * 64],
                                            in0=pT[:], in1=hi, op=ALU.subtract)
                tabv = tab.rearrange("(c p) ch -> p c ch", p=128)
                hiv = NT_hi[:].rearrange("p (c ch) -> p c ch", ch=64)
                lov = NT_lo[:].rearrange("p (c ch) -> p c ch", ch=64)
                nc.sync.dma_start(out=tabv[:, :, 0:64], in_=hiv)
                nc.sync.dma_start(out=tabv[:, :, 64:128], in_=lov)

            build_ac(A_sb, w1sum_sb, "A_ps")
            build_ac(C1_sb, w1b_sb, "C1_ps")
            build_ac(C2_sb, w21p_sb, "C2_ps")
            build_table(A_sb, a_tab, "aT")
            ck(3)

            # ---------------- kNN top-16 ----------------
            # score[i,j] = 2 p_i.p_j - |p_j|^2  (row-constant -|p_i|^2 dropped)
            # idxw is built per-HALF so conv1 gathers for the first 512 nodes
            # can start while the second half of the top-k still runs:
            # u32 idx -> f32 (exact) -> PE transpose -> i16 -> DRAM bounce ->
            # one strided read per gpsimd core group.
            for m in range(8):
                D_ps = ps_big.tile([128, N], F32, name="big")
                for h in range(2):
                    sl = slice(h * 512, (h + 1) * 512)
                    nc.tensor.matmul(out=D_ps[:, sl],
                                     lhsT=lhsT4[:, m * 128:(m + 1) * 128],
                                     rhs=rhs4[:, sl], start=True, stop=True)
                D_sb = scr.tile([128, N], F32, name="dsb")
                nc.scalar.copy(D_sb[:], D_ps[:])
                nc.vector.max(v8a[:], D_sb[:])
                nc.vector.max_index(packed32[:, m * 16:m * 16 + 8], v8a[:], D_sb[:])
                nc.vector.match_replace(D_sb[:], v8a[:], D_sb[:], -1e30)
                nc.vector.max(v8b[:], D_sb[:])
                nc.vector.max_index(packed32[:, m * 16 + 8:m * 16 + 16], v8b[:],
                                    D_sb[:])
                if m % 4 == 3:
                    hb = m // 4  # half 0: chunks 0-3, half 1: chunks 4-7
                    cs = slice(hb * 64, (hb + 1) * 64)
                    nc.scalar.copy(pf32[:], packed32[:, cs].bitcast(mybir.dt.int32))
                    pT2 = ps_small.tile([64, 128], F32, name="small")
                    nc.tensor.transpose(pT2[:], pf32[:], ident[:])
                    nc.scalar.copy(TtI16[:], pT2[:])
                    nc.sync.dma_start(out=pk_d[cs, :], in_=TtI16[:])
                    for g in range(8):
                        nc.sync.dma_start(
                            out=idxw[g * 16:(g + 1) * 16,
                                     hb * 512:(hb + 1) * 512].rearrange(
                                         "k (m p) -> k m p", m=4),
                            in_=pk_d[cs, :].rearrange("(m k) p -> k m p", k=16))
            ck(2)

            def fold_one(et, h_t, c_sb):
                g = gats[et]
                sl = slice(et * 512, (et + 1) * 512)
                s_ps = ps_small.tile([64, 512], F32, name="small")
                nc.tensor.matmul(out=s_ps[:], lhsT=fold[:], rhs=g[:],
                                 start=True, stop=True)
                nc.vector.tensor_tensor(
                    out=h_t[:, sl].rearrange("c (i k) -> c i k", k=16),
                    in0=s_ps[:].rearrange("c (i k) -> c i k", k=16),
                    in1=c_sb[:, et * 32:(et + 1) * 32].to_broadcast([64, 32, 16]),
                    op=ALU.subtract)
                # BN stats interleave with the gather stream instead of
                # serializing 21us of vector work after it
                nc.vector.bn_stats(bnst[0:64, et * 6:(et + 1) * 6], h_t[:, sl])

            def gather_sub(tab, h_t, c_sb):
                # hw limit: <=512 idxs per dma_gather call; per-chunk tiles so
                # chunk c+1's DMA overlaps chunk c's fold matmul
                for et in range(32):
                    g = gats[et]
                    nc.gpsimd.dma_gather(
                        out_ap=g[:].rearrange("p (one e) -> p one e", one=1),
                        in_ap=tab[:],
                        idxs_ap=idxw[:, et * 32:(et + 1) * 32], num_idxs=512,
                        num_idxs_reg=512, elem_size=128, transpose=True,
                        queue_num=et % 4)
                    fold_one(et, h_t, c_sb)



            def bn_relu(h_t, out_t, P, gb_sb, cci, cco):
                # global-batch BN (AllReduce mean/E[x^2]), then relu' =
                # relu(x + b/a) split across scalar AND vector; the a-scale
                # folds into the NEXT layer's weights (a = gamma/sigma > 0)
                nc.vector.bn_aggr(st2[0:P, :], bnst[0:P, :])
                nc.scalar.copy(st_in[0:P, 0:1], st2[0:P, 0:1])
                nc.scalar.square(st_in[0:P, 1:2], st2[0:P, 0:1])
                nc.vector.tensor_tensor(out=st_in[0:P, 1:2], in0=st_in[0:P, 1:2],
                                        in1=st2[0:P, 1:2], op=ALU.add)
                nc.sync.dma_start(out=cci[:], in_=st_in[0:P, :])
                nc.gpsimd.collective_compute("AllReduce", ALU.add,
                                             replica_groups=RG,
                                             ins=[cci.opt()], outs=[cco.opt()])
                nc.sync.dma_start(out=st_g[0:P, :], in_=cco[:])
                _bn_coeffs(P, gb_sb)
                nc.vector.reciprocal(bnw[0:P, 8:9], bnw[0:P, 6:7])
                nc.vector.tensor_tensor(out=bnw[0:P, 8:9], in0=bnw[0:P, 7:8],
                                        in1=bnw[0:P, 8:9], op=ALU.mult)  # b/a
                for c in range(8):
                    sl = slice(c * 2048, (c + 1) * 2048)
                    if c % 2 == 0:
                        nc.scalar.activation(out_t[:, sl], h_t[:, sl], AF.Relu,
                                             bias=bnw[0:P, 8:9], scale=1.0)
                    else:
                        nc.vector.tensor_scalar(
                            out=out_t[:, sl], in0=h_t[:, sl],
                            scalar1=bnw[0:P, 8:9], scalar2=0.0,
                            op0=ALU.add, op1=ALU.max)

            def scale_rows(dst, src, P):
                # dst = diag(a) @ src, folding the BN scale into the next
                # layer's stationary weights
                nc.vector.tensor_tensor(
                    out=dst[:], in0=src[:],
                    in1=bnw[0:P, 6:7].to_broadcast(list(src.shape)),
                    op=ALU.mult)

            def _bn_coeffs(P, gb_sb):
                nc.scalar.mul(bnw[0:P, 0:1], st_g[0:P, 0:1], 1.0 / NCORES)  # m
                nc.scalar.mul(bnw[0:P, 1:2], st_g[0:P, 1:2], 1.0 / NCORES)  # q
                nc.scalar.square(bnw[0:P, 2:3], bnw[0:P, 0:1])
                nc.vector.tensor_tensor(out=bnw[0:P, 3:4], in0=bnw[0:P, 1:2],
                                        in1=bnw[0:P, 2:3], op=ALU.subtract)  # var
                nc.scalar.activation(bnw[0:P, 4:5], bnw[0:P, 3:4], AF.Sqrt,
                                     bias=epsc[0:P, 0:1], scale=1.0)
                nc.vector.reciprocal(bnw[0:P, 5:6], bnw[0:P, 4:5])
                nc.vector.tensor_tensor(out=bnw[0:P, 6:7], in0=gb_sb[0:P, 0:1],
                                        in1=bnw[0:P, 5:6], op=ALU.mult)  # scale
                nc.vector.tensor_tensor(out=bnw[0:P, 8:9], in0=bnw[0:P, 0:1],
                                        in1=bnw[0:P, 6:7], op=ALU.mult)
                nc.vector.tensor_tensor(out=bnw[0:P, 7:8], in0=gb_sb[0:P, 1:2],
                                        in1=bnw[0:P, 8:9], op=ALU.subtract)  # bias

            # ---------------- conv1 ----------------
            h1 = hA[0:64, :]
            gather_sub(a_tab, h1, C1_sb)
            ck(5)
            relu1 = hB[0:64, :]
            bn_relu(h1, relu1, 64, gb1_sb, cc1i, cc1o)
            scale_rows(w2s, w2_sb, 64)
            ck(6)
            for et in range(32):
                sl = slice(et * 512, (et + 1) * 512)
                ps = ps_small.tile([64, 512], F32, name="small")
                nc.tensor.matmul(out=ps[:], lhsT=w2s[:], rhs=relu1[:, sl],
                                 start=True, stop=True)
                nc.vector.tensor_reduce(
                    out=x1m[:, et * 32:(et + 1) * 32],
                    in_=ps[:].rearrange("c (i k) -> c i k", k=16),
                    axis=AX.X, op=ALU.max)

            ck(7)
            # ---------------- conv2 ----------------
            nc.scalar.copy(x1hi[:], x1m[:])
            nc.vector.tensor_tensor(out=x1lo[:], in0=x1m[:], in1=x1hi[:],
                                    op=ALU.subtract)
            G2_ps = ps_big.tile([64, N], F32, name="big")
            for h in range(2):
                sl = slice(h * 512, (h + 1) * 512)
                nc.tensor.matmul(out=G2_ps[:, sl], lhsT=w21x_sb[:],
                                 rhs=x1hi[:, sl], start=True, stop=False)
                nc.tensor.matmul(out=G2_ps[:, sl], lhsT=w21x_sb[:],
                                 rhs=x1lo[:, sl], start=False, stop=False)
                nc.tensor.matmul(out=G2_ps[:, sl], lhsT=w21p16_sb[:],
                                 rhs=pos16[:, sl], start=False, stop=False)
                nc.tensor.matmul(out=G2_ps[:, sl], lhsT=w21p16_sb[:],
                                 rhs=poslo[:, sl], start=False, stop=True)
            nc.scalar.copy(G2_sb[:], G2_ps[:])
            build_table(G2_sb, g2_tab, "gT")
            ck(8)

            h21 = hA[0:64, :]
            gather_sub(g2_tab, h21, C2_sb)
            relu21 = hB[0:64, :]
            bn_relu(h21, relu21, 64, gb21_sb, cc2i, cc2o)

            scale_rows(w22s, w22_sb, 64)
            h22 = hA[:]
            for et in range(32):
                sl = slice(et * 512, (et + 1) * 512)
                ps = ps_small.tile([128, 512], F32, name="small")
                nc.tensor.matmul(out=ps[:], lhsT=w22s[:], rhs=relu21[:, sl],
                                 start=True, stop=True)
                nc.scalar.copy(h22[:, sl], ps[:])
                nc.vector.bn_stats(bnst[:, et * 6:(et + 1) * 6], h22[:, sl])
            relu22 = hB[:]
            bn_relu(h22, relu22, 128, gb22_sb, cc3i, cc3o)
            scale_rows(w3s, w3_sb, 128)
            ck(9)

            # ---------------- conv2 L3 + edge-max pool ----------------
            # [128,1024] 2-bank PSUM chunks, max-reduced straight from PSUM.
            # The g AllGather + final-linear accumulation are split in two so
            # the first half overlaps the second half of the L3 matmuls.
            def ag_issue(h2):
                # reduce + kick the AllGather for oc half h2; consumers of
                # the gathered data come later so the ~15us collective
                # latency hides under the remaining L3 matmuls
                for oc in range(h2 * 4, h2 * 4 + 4):
                    nc.vector.tensor_reduce(
                        out=g_sb[:, oc:oc + 1],
                        in_=gc8[:, oc * 128:(oc + 1) * 128],
                        axis=AX.X, op=ALU.max)
                nc.sync.dma_start(
                    out=g_inh[h2].rearrange("(c p) -> p c", p=128),
                    in_=g_sb[:, h2 * 4:(h2 + 1) * 4])
                nc.gpsimd.collective_compute("AllGather", ALU.bypass,
                                             replica_groups=RG,
                                             ins=[g_inh[h2].opt()],
                                             outs=[g_outh[h2].opt()])
                nc.sync.dma_start(out=go_sb[:, h2 * 512:(h2 + 1) * 512],
                                  in_=g_outh[h2][:])

            def ag_consume(h2):
                for cl in range(4):
                    c = h2 * 4 + cl
                    pT = ps_big.tile([128, 8], F32, name="big")
                    nc.tensor.transpose(
                        pT[:], go_sb[:, c * 128:(c + 1) * 128],
                        ident[0:8, 0:8])
                    nc.scalar.copy(gall_sb[:, c * 8:(c + 1) * 8], pT[:])

            for oc in range(8):
                for eb in range(16):
                    ps = ps_big.tile([128, 1024], F32, name="big")
                    for h in range(2):
                        sl = slice(eb * 1024 + h * 512,
                                   eb * 1024 + (h + 1) * 512)
                        nc.tensor.matmul(out=ps[:, h * 512:(h + 1) * 512],
                                         lhsT=w3s[:, oc * 128:(oc + 1) * 128],
                                         rhs=relu22[:, sl],
                                         start=True, stop=True)
                    col = oc * 16 + eb
                    nc.vector.tensor_reduce(
                        out=gc8[:, col * 8:(col + 1) * 8],
                        in_=ps[:].rearrange("p (j e) -> p j e", j=8),
                        axis=AX.X, op=ALU.max)
                if oc == 3:
                    ag_issue(0)

            ck(10)
            ag_issue(1)
            psfs = []
            ag_consume(0)
            for oc2 in range(2):
                psf = ps_small.tile([128, 8], F32, name="small")
                psfs.append(psf)
                for kc in range(4):
                    base = kc * 256 + oc2 * 128
                    nc.tensor.matmul(out=psf[:],
                                     lhsT=linw_sb[:, base:base + 128],
                                     rhs=gall_sb[:, kc * 8:(kc + 1) * 8],
                                     start=(kc == 0), stop=False,
                                     skip_group_check=True)
            ag_consume(1)
            for oc2 in range(2):
                psf = psfs[oc2]
                for kc in range(4, 8):
                    base = kc * 256 + oc2 * 128
                    nc.tensor.matmul(out=psf[:],
                                     lhsT=linw_sb[:, base:base + 128],
                                     rhs=gall_sb[:, kc * 8:(kc + 1) * 8],
                                     start=False, stop=(kc == 7),
                                     skip_group_check=True)

            ck(11)
            # ---------------- final linear + local BN + relu ----------------
            for oc2 in range(2):
                psf = psfs[oc2]
                nc.vector.tensor_reduce(out=bnw[:, 9:10], in_=psf[:],
                                        axis=AX.X, op=ALU.add)
                nc.scalar.mul(bnw[:, 0:1], bnw[:, 9:10], 1.0 / B)  # m
                nc.scalar.activation(qdump[:], psf[:], AF.Square,
                                     accum_out=bnw[:, 9:10])
                nc.scalar.mul(bnw[:, 1:2], bnw[:, 9:10], 1.0 / B)  # q
                nc.scalar.square(bnw[:, 2:3], bnw[:, 0:1])
                nc.vector.tensor_tensor(out=bnw[:, 3:4], in0=bnw[:, 1:2],
                                        in1=bnw[:, 2:3], op=ALU.subtract)
                nc.scalar.activation(bnw[:, 4:5], bnw[:, 3:4], AF.Sqrt,
                                     bias=epsc[:], scale=1.0)
                nc.vector.reciprocal(bnw[:, 5:6], bnw[:, 4:5])
                nc.vector.tensor_tensor(out=bnw[:, 6:7],
                                        in0=gbf_sb[:, oc2:oc2 + 1],
                                        in1=bnw[:, 5:6], op=ALU.mult)
                nc.vector.tensor_tensor(out=bnw[:, 8:9], in0=bnw[:, 0:1],
                                        in1=bnw[:, 6:7], op=ALU.mult)
                nc.vector.tensor_tensor(out=bnw[:, 7:8],
                                        in0=gbf_sb[:, 2 + oc2:3 + oc2],
                                        in1=bnw[:, 8:9], op=ALU.subtract)
                nc.scalar.activation(outf_sb[:], psf[:], AF.Relu,
                                     bias=bnw[:, 7:8], scale=bnw[:, 6:7])
                pso = ps_big.tile([8, 128], F32, name="big")
                nc.tensor.transpose(pso[:], outf_sb[:], ident[:])
                nc.scalar.copy(osb[:], pso[:])
                nc.sync.dma_start(out=out_d[:, oc2 * 128:(oc2 + 1) * 128],
                                  in_=osb[:])

    except _Stop:
        pass
    for f in reversed(frees):
        f()


def _build():
    nc = bacc.Bacc("TRN2", target_bir_lowering=False, debug=False,
                   num_devices=NCORES, num_swdge_queues=4)
    d = {}

    def inp(name, shape, dtype):
        d[name] = nc.dram_tensor(name, shape, dtype, kind="ExternalInput").ap()

    inp("posT", [3, N], F32)
    inp("w1sum", [3, 64], F32)
    inp("w1b", [3, 64], F32)
    inp("w21p", [3, 64], F32)
    inp("w2w", [64, 64], F16)
    inp("w21x", [64, 64], F16)
    inp("w21p16", [3, 64], F16)
    inp("w22w", [64, 128], F16)
    inp("w3w", [128, 1024], F16)
    inp("linw", [1024, 256], F32)
    inp("gb1", [64, 2], F32)
    inp("gb21", [64, 2], F32)
    inp("gb22", [128, 2], F32)
    inp("gbf", [128, 4], F32)
    out_d = nc.dram_tensor("out", [B, 256], F32, kind="ExternalOutput").ap()

    with tile.TileContext(nc) as tc:
        _emit(nc, tc, d, out_d)
    nc.finalize()
    return nc


_NC = None


def _get_nc():
    global _NC
    if _NC is None:
        _NC = _build()
    return _NC


def _prepare_in_maps(inputs):
    f32 = np.float32
    f16 = np.float16
    pos = np.asarray(inputs["pos"], dtype=f32)
    c1_W1 = np.asarray(inputs["c1_W1"], dtype=f32)
    c2_W1 = np.asarray(inputs["c2_W1"], dtype=f32)
    common = {
        "w1sum": np.ascontiguousarray(c1_W1[0:3] + c1_W1[3:6]),
        "w1b": np.ascontiguousarray(c1_W1[3:6]),
        "w21p": np.ascontiguousarray(c2_W1[64:67]),
        "w2w": np.asarray(inputs["c1_W2"], dtype=f16),
        "w21x": np.ascontiguousarray(c2_W1[0:64].astype(f16)),
        "w21p16": np.ascontiguousarray(c2_W1[64:67].astype(f16)),
        "w22w": np.asarray(inputs["c2_W2"], dtype=f16),
        "w3w": np.asarray(inputs["c2_W3"], dtype=f16),
        "linw": np.asarray(inputs["lin_W"], dtype=f32),
        "gb1": np.ascontiguousarray(
            np.stack([inputs["c1_g1"], inputs["c1_be1"]], axis=1).astype(f32)),
        "gb21": np.ascontiguousarray(
            np.stack([inputs["c2_g1"], inputs["c2_be1"]], axis=1).astype(f32)),
        "gb22": np.ascontiguousarray(
            np.stack([inputs["c2_g2"], inputs["c2_be2"]], axis=1).astype(f32)),
        "gbf": np.ascontiguousarray(np.stack(
            [np.asarray(inputs["lin_g"], dtype=f32)[0:128],
             np.asarray(inputs["lin_g"], dtype=f32)[128:256],
             np.asarray(inputs["lin_be"], dtype=f32)[0:128],
             np.asarray(inputs["lin_be"], dtype=f32)[128:256]], axis=1)),
    }
    in_maps = []
    for c in range(NCORES):
        m = dict(common)
        m["posT"] = np.ascontiguousarray(pos[c * N:(c + 1) * N].T)
        in_maps.append(m)
    return in_maps


def _run(inputs, trace=False, **kw):
    return bass_utils.run_bass_kernel_spmd(
        _get_nc(), _prepare_in_maps(inputs),
        core_ids=list(range(NCORES)), trace=trace, **kw)


def kernel(**inputs):
    res = _run(inputs)
    return np.asarray(res.results[0]["out"], dtype=np.float32)

